# revision 19
# baseline (speedup 1.0000x reference)
"""Cross-attention (efficient-attention variant) + 1x1 conv + LayerNorm on 8 trn2 cores.

Problem: x1,x2 [4,64,64,1024] f32. Per batch b and head h (8 heads, 128 ch each):
  value = x1[b] channel-major, kq = x2[b] channel-major
  key = softmax(kq, tokens), query = softmax(kq, head-channels)
  S = query @ key^T  [128,128];  att = S @ value  -> agg [1024, 4096]
  y = w_proj[2048,1024] @ agg + b_proj; LayerNorm(2048) * gamma + beta

Sharding: core i -> batch b=i//2, token half i%2 (2048 tokens).

Algebra (validated vs reference in f64):
  E = exp(x2);  qs[n,h] = sum_{c in h} E[n,c];  Qh = E / qs  (per head)
  st_h = E_h^T @ Qh_h  (symmetric == E^T diag(1/qs) E);  Z = rowsum(st_h)
  (rowsum(st_h) == colsum(E_h) exactly: the per-token normalizers cancel)
  stn_h = diag(1/Z) st_h  (applied in the PSUM->SBUF copy via ACT scale=)
  W~_h = stn_h @ wTc_h  where wTc = wT - rowmean(wT)  (HOST-centered: makes
  z' = sum_h V_h^T W~_h == z - mean(z) exactly -> no mean pass on device)
  LayerNorm: var = sum(z'^2)/2C (scalar Square+accum from PSUM), out =
  z' * rsqrt(var+eps) (vector tensor_scalar PSUM->SBUF bf16). gamma/beta,
  and the final f32 cast, on host.

Phase A is elementwise-bound: exp on scalar, per-head reduce on vector,
Qh mult one broadcast TT per tile on gpsimd (K_G tiles) or vector (rest).
Phase B is pure proj matmuls: lhsT = V tiles straight from DMA (att never
materialized), rhs = W~.  Tensor-bound ~109us.
"""

import os
import numpy as np

import concourse.bass as bass
import concourse.tile as tile
from concourse import bacc, mybir
from concourse.bass_utils import run_bass_kernel_spmd

F32 = mybir.dt.float32
BF16 = mybir.dt.bfloat16
AX = mybir.AxisListType
ALU = mybir.AluOpType
ACT_F = mybir.ActivationFunctionType

B, HI, WI, C = 4, 64, 64, 1024
N = HI * WI          # 4096 tokens per batch
HEADS = 8
CH = C // HEADS      # 128 per-head channels
C2 = 2 * C           # 2048 output channels
NCORES = 8
TOK = N // 2         # 2048 tokens per core (phase B)
P = 128
NT_A = N // P        # 32 token tiles in phase A
CHUNK = 512          # phase-B V chunk (tokens)
NCHUNK = TOK // CHUNK
NSUB = CHUNK // P    # 4 subs per chunk
OC = C2 // 512       # output-channel chunks of 512
HC = C2 // 2         # 1024: half-row granularity for PSUM tiles
EPS = 1e-5

K_G = int(os.environ.get("K_G", "28"))   # tiles whose Qh mult runs on gpsimd

_compiled = {}


def _spread(total, count):
    if count <= 0:
        return [False] * total
    s = {int(i * total / count) for i in range(count)}
    return [i in s for i in range(total)]


def build(has_bias=False):
    nc = bacc.Bacc("TRN2", target_bir_lowering=False, debug=False,
                   num_devices=NCORES)
    xq = nc.dram_tensor("xq", [N, C], BF16, kind="ExternalInput").ap()
    vcm = nc.dram_tensor("vcm", [C, TOK], BF16, kind="ExternalInput").ap()
    wt = nc.dram_tensor("wt", [C, C2], BF16, kind="ExternalInput").ap()
    if has_bias:
        brep = nc.dram_tensor("brep", [P, C2], F32, kind="ExternalInput").ap()
    y = nc.dram_tensor("y", [TOK, C2], BF16, kind="ExternalOutput").ap()

    g_tile = _spread(NT_A, K_G)

    with tile.TileContext(nc) as tc:
        with tc.tile_pool(name="persist", bufs=1) as persist:
            eps_sb = persist.tile([P, 1], F32, name="eps")
            nc.vector.memset(eps_sb[:], EPS)
            wt_sb = [persist.tile([P, C2], BF16, name=f"wt{h}") for h in range(HEADS)]
            for h in range(HEADS):
                nc.sync.dma_start(wt_sb[h][:], wt[h * P:(h + 1) * P, :])
            if has_bias:
                brep_sb = persist.tile([P, C2], F32, name="brep")
                nc.sync.dma_start(brep_sb[:], brep[:])
            st_sb = [persist.tile([P, CH], BF16, name=f"st{h}") for h in range(HEADS)]
            wl_sb = [persist.tile([P, C2], BF16, name=f"wl{h}") for h in range(HEADS)]
            zz = persist.tile([P, HEADS], F32, name="zz")
            rz = persist.tile([P, HEADS], F32, name="rz")

            # ---------------- Phase A: st_h = E_h^T Qh_h over N tokens --------
            with tc.tile_pool(name="xq_p", bufs=6) as xq_p, \
                 tc.tile_pool(name="e_p", bufs=8) as e_p, \
                 tc.tile_pool(name="et_p", bufs=8) as et_p, \
                 tc.tile_pool(name="sm_a", bufs=16) as sm_a, \
                 tc.tile_pool(name="st_ps", bufs=1, space="PSUM") as st_psp:
                st_ps = [st_psp.tile([P, CH], F32, name=f"stp{h}") for h in range(HEADS)]
                for nt in range(NT_A):
                    xt = xq_p.tile([P, C], BF16)
                    nc.sync.dma_start(xt[:], xq[nt * P:(nt + 1) * P, :])
                    E = e_p.tile([P, C], BF16)
                    nc.scalar.activation(E[:], xt[:], ACT_F.Exp)
                    qs = sm_a.tile([P, HEADS], F32, name="qs")
                    nc.vector.reduce_sum(
                        qs[:], E.rearrange("p (h c) -> p h c", h=HEADS),
                        axis=AX.X)
                    rq = sm_a.tile([P, HEADS], F32, name="rq")
                    nc.vector.reciprocal(rq[:], qs[:])
                    Qh = et_p.tile([P, C], BF16)
                    if g_tile[nt]:
                        nc.gpsimd.tensor_tensor(
                            Qh.rearrange("p (h c) -> p h c", h=HEADS),
                            E.rearrange("p (h c) -> p h c", h=HEADS),
                            rq[:, :, None].to_broadcast([P, HEADS, CH]),
                            op=ALU.mult)
                    else:
                        for h in range(HEADS):
                            hs = slice(h * CH, (h + 1) * CH)
                            nc.scalar.activation(Qh[:, hs], E[:, hs],
                                                 ACT_F.Copy,
                                                 scale=rq[:, h:h + 1])
                    first, last = nt == 0, nt == NT_A - 1
                    for h in range(HEADS):
                        hs = slice(h * CH, (h + 1) * CH)
                        nc.tensor.matmul(st_ps[h][:], lhsT=E[:, hs],
                                         rhs=Qh[:, hs], start=first, stop=last)
                # Z = rowsum(st_raw); stn = diag(1/Z) st -> SBUF bf16
                for h in range(HEADS):
                    nc.vector.reduce_sum(zz[:, h:h + 1], st_ps[h][:], axis=AX.X)
                    nc.vector.reciprocal(rz[:, h:h + 1], zz[:, h:h + 1])
                    nc.scalar.copy(st_sb[h][:], st_ps[h][:])

            # ---------------- W~_h = stn_h wTc_h  [128, 2048] ------------------
            with tc.tile_pool(name="wl_ps", bufs=3, space="PSUM") as wl_psp:
                for h in range(HEADS):
                    for half in range(2):
                        wps = wl_psp.tile([P, HC], F32, tag="wps",
                                          name=f"wps{h}_{half}")
                        for oc in range(2):
                            sl = slice(oc * 512, (oc + 1) * 512)
                            gl = slice(half * HC + oc * 512,
                                       half * HC + (oc + 1) * 512)
                            nc.tensor.matmul(wps[:, sl], lhsT=st_sb[h][:],
                                             rhs=wt_sb[h][:, gl],
                                             start=True, stop=True)
                        dst = wl_sb[h][:, half * HC:(half + 1) * HC]
                        if half == 0:
                            nc.scalar.activation(dst, wps[:], ACT_F.Copy,
                                                 scale=rz[:, h:h + 1])
                        else:
                            nc.vector.tensor_scalar_mul(dst, wps[:],
                                                        rz[:, h:h + 1])

            # ---------------- Phase B: z' = sum_h V_h^T W~_h, LayerNorm -------
            with tc.tile_pool(name="v_p", bufs=4) as v_p, \
                 tc.tile_pool(name="yn_p", bufs=3) as yn_p, \
                 tc.tile_pool(name="sqs_p", bufs=4) as sqs_p, \
                 tc.tile_pool(name="ysb_p", bufs=2) as ysb_p, \
                 tc.tile_pool(name="sm_b", bufs=10) as sm_b, \
                 tc.tile_pool(name="ps_b", bufs=8, space="PSUM") as ps_b:
                vcm_r = vcm.rearrange("(h p) n -> p h n", p=P)
                for ck in range(NCHUNK):
                    vt = v_p.tile([P, HEADS * CHUNK], BF16)
                    nc.sync.dma_start(
                        vt.rearrange("p (h n) -> p h n", h=HEADS),
                        vcm_r[:, :, ck * CHUNK:(ck + 1) * CHUNK])
                    for sub in range(NSUB):
                        yps = [ps_b.tile([P, 512], F32, tag="yps",
                                         name=f"yps{ck}_{sub}_{k}")
                               for k in range(OC)]
                        for h in range(HEADS):
                            lt = vt[:, h * CHUNK + sub * P:
                                    h * CHUNK + (sub + 1) * P]
                            st_, sp_ = h == 0, h == HEADS - 1
                            for oc in range(OC):
                                nc.tensor.matmul(
                                    yps[oc][:], lhsT=lt,
                                    rhs=wl_sb[h][:, oc * 512:(oc + 1) * 512],
                                    start=st_, stop=sp_)
                        # ---- LN: var = sum(z'^2)/2C ----
                        ssq4 = sm_b.tile([P, OC], F32, name="ssq4")
                        if has_bias:
                            ysb = ysb_p.tile([P, C2], F32)
                            for k in range(OC):
                                sl = slice(k * 512, (k + 1) * 512)
                                nc.vector.tensor_tensor(
                                    ysb[:, sl], yps[k][:], brep_sb[:, sl],
                                    op=ALU.add)
                                nc.scalar.activation(
                                    sqs_p.tile([P, 512], BF16, name="sqs"),
                                    ysb[:, sl], ACT_F.Square,
                                    accum_out=ssq4[:, k:k + 1])
                        else:
                            for k in range(OC):
                                nc.scalar.activation(
                                    sqs_p.tile([P, 512], BF16, name="sqs"),
                                    yps[k][:], ACT_F.Square,
                                    accum_out=ssq4[:, k:k + 1])
                        ss = sm_b.tile([P, 1], F32, name="ss")
                        nc.vector.reduce_sum(ss[:], ssq4[:], axis=AX.X)
                        var = sm_b.tile([P, 1], F32, name="var")
                        nc.vector.tensor_scalar_mul(var[:], ss[:], 1.0 / C2)
                        sig = sm_b.tile([P, 1], F32, name="sig")
                        nc.scalar.activation(sig[:], var[:], ACT_F.Sqrt,
                                             bias=eps_sb[:])
                        rsig = sm_b.tile([P, 1], F32, name="rsig")
                        nc.vector.reciprocal(rsig[:], sig[:])
                        # ---- normalize: z' * rsig, PSUM -> SBUF bf16 ----
                        yn = yn_p.tile([P, C2], BF16)
                        for k in range(OC):
                            sl = slice(k * 512, (k + 1) * 512)
                            srcap = ysb[:, sl] if has_bias else yps[k][:]
                            nc.vector.tensor_scalar_mul(yn[:, sl], srcap,
                                                        rsig[:])
                        row = (ck * NSUB + sub) * P
                        nc.sync.dma_start(y[row:row + P, :], yn[:])
    nc.compile()
    return nc


def _get_nc(has_bias):
    key = (has_bias, K_G)
    if key not in _compiled:
        _compiled[key] = build(has_bias)
    return _compiled[key]


def run(inputs, trace=False):
    import ml_dtypes
    x1 = np.asarray(inputs["x1"], dtype=np.float32)
    x2 = np.asarray(inputs["x2"], dtype=np.float32)
    w_proj = np.asarray(inputs["w_proj"], dtype=np.float32)
    b_proj = np.asarray(inputs["b_proj"], dtype=np.float32)
    gamma = np.asarray(inputs["gamma"], dtype=np.float32)
    beta = np.asarray(inputs["beta"], dtype=np.float32)

    has_bias = bool(np.any(b_proj != 0.0))
    x1f = x1.reshape(B, N, C)
    x2f = x2.reshape(B, N, C).astype(ml_dtypes.bfloat16)
    wtr = np.ascontiguousarray(w_proj.T)                     # [C, 2C]
    wtc = (wtr - wtr.mean(axis=1, keepdims=True)).astype(ml_dtypes.bfloat16)

    in_maps = []
    for core in range(NCORES):
        b, half = divmod(core, 2)
        vcm = np.ascontiguousarray(
            x1f[b].T[:, half * TOK:(half + 1) * TOK]).astype(ml_dtypes.bfloat16)
        m = {"xq": np.ascontiguousarray(x2f[b]), "vcm": vcm, "wt": wtc}
        if has_bias:
            bc = b_proj - b_proj.mean()
            m["brep"] = np.ascontiguousarray(
                np.broadcast_to(bc, (P, C2))).astype(np.float32)
        in_maps.append(m)
    nc = _get_nc(has_bias)
    res = run_bass_kernel_spmd(nc, in_maps, list(range(NCORES)), trace=trace)

    yout = np.empty((B, N, C2), np.float32)
    for core in range(NCORES):
        b, half = divmod(core, 2)
        yout[b, half * TOK:(half + 1) * TOK] = res.results[core]["y"].astype(
            np.float32)
    yout = yout * gamma + beta
    return yout.reshape(B, HI, WI, C2), res


def kernel(**inputs):
    out, _ = run(inputs, trace=False)
    return out
